# revision 4
# baseline (speedup 1.0000x reference)
"""CuGraphSAGE (3x SAGE conv + linear) on 8 Trainium2 NeuronCores.

Strategy: partition destination nodes across the 8 cores (vertex-cut),
replicate the full node-feature table on every core (rebuilt each layer via
AllGather), and compute each layer's mean-aggregation with:
  - dma_gather (custom Q7 SWDGE gather) of source rows, int16 indices over
    4 source-row chunks (int16 range limit),
  - per-128-edge-group selection matrices S (built on DVE:
    S[p, j] = (j == dstloc[p]) * invdeg[p]) and PE matmuls
    psum[feat, dst] += msgs_g.T @ S_g  (segment-sum + mean fold),
  - dense layer z.T = W_top.T @ aggT + W_bot.T @ hT, fused ReLU(+bias) on
    ACT, dropout via precomputed host masks (exact jax threefry bits).

The kernel is SPMD: one NEFF for all 8 cores, so the static group structure
(groups per (chunk, tile)) is the max over cores; per-core slack is padded
with dst=-1 slots that contribute nothing.
"""

import os
import time

import numpy as np

P = 128
NCORES = 8
N = 100000
E = 1600000
IN = 128
HID = 128
OUT = 64
DROP_P = 0.5
OWN = N // NCORES            # 12500 dst nodes per core
TW = 512                     # aggregation tile width (dst nodes per PSUM tile)
TILES = (OWN + TW - 1) // TW  # 25 aggregation tiles
OWNP = TILES * TW            # 12800 padded dst slots per core
DTILES = OWNP // P           # 100 dense-phase column tiles
NPAD = NCORES * OWNP         # 102400 padded node table rows
NCHUNK = 4
CHROWS = NPAD // NCHUNK      # 25600 (< 32768, int16-safe)
GCALL = 8                    # groups per dma_gather call (1024 rows; SWDGE ring carveout limit)

_F32 = None  # set lazily (mybir import is heavy)


# ---------------------------------------------------------------------------
# host-side preprocessing
# ---------------------------------------------------------------------------

def _dropout_masks():
    """Exact reproduction of the reference's dropout draws (jax threefry is
    platform-deterministic; run on CPU so the axon/neuron backend is never
    touched)."""
    import jax
    import jax.numpy as jnp

    cpu = jax.devices("cpu")[0]
    with jax.default_device(cpu):
        dkey = jax.random.key(42)
        masks = []
        for i in range(3):
            keep = jax.random.bernoulli(
                jax.random.fold_in(dkey, i), 1.0 - DROP_P, (N, HID))
            masks.append(np.asarray(keep, dtype=np.float32) * 2.0)
    return masks  # each [N, HID] with values {0.0, 2.0}


def _preprocess(edge):
    """Build per-core gather/S data and the shared static structure.

    Returns dict with:
      perm[c]   : [OWN] node order within core c (degree-sorted); position
                  j in the padded layout holds original node c*OWN+perm[c][j]
      pos_glob  : [N] padded-layout row of each original node
      ngroups   : [NCHUNK, TILES] static groups per (chunk, tile)
      spans     : list over chunks of list of (tile, ngroups) in emit order
      idx16[c]  : [128, SLOTS//16] int16 gather indices (chunk-relative)
      dst32[c]  : [128, GTOT] f32 within-tile dst (or -1 for pad)
      inv32[c]  : [128, GTOT] f32 1/max(deg,1) of the slot's dst
    """
    src, dst = edge[0].astype(np.int64), edge[1].astype(np.int64)
    deg = np.bincount(dst, minlength=N)
    invdeg = (1.0 / np.maximum(deg, 1)).astype(np.float32)

    perms = []
    pos_glob = np.empty(N, np.int64)
    for c in range(NCORES):
        d = deg[c * OWN:(c + 1) * OWN]
        order = np.argsort(d, kind="stable")
        perms.append(order)
        pos = np.empty(OWN, np.int64)
        pos[order] = np.arange(OWN)
        pos_glob[c * OWN:(c + 1) * OWN] = c * OWNP + pos
    src_r = pos_glob[src]          # padded-layout row of the source node
    ch = src_r // CHROWS
    idx_in_chunk = src_r - ch * CHROWS

    core = dst // OWN
    dstpos = pos_glob[dst] - core * OWNP    # position within core, 0..OWNP-1
    tileid = dstpos // TW

    # per-core (chunk, tile) edge lists
    counts = np.zeros((NCORES, NCHUNK, TILES), np.int64)
    np.add.at(counts, (core, ch, tileid), 1)
    ngroups = np.ceil(counts.max(axis=0) / P).astype(np.int64)  # [NCHUNK,TILES]
    gtot = int(ngroups.sum())
    slots = gtot * P

    # slot offsets per (chunk, tile) in emit order (chunk-major, tile within)
    off = np.zeros((NCHUNK, TILES), np.int64)
    acc = 0
    spans = []
    for c_ in range(NCHUNK):
        row = []
        for t in range(TILES):
            off[c_, t] = acc
            acc += ngroups[c_, t] * P
            if ngroups[c_, t] > 0:
                row.append((t, int(ngroups[c_, t])))
        spans.append(row)
    assert acc == slots

    idx16, dst32, inv32 = [], [], []
    for c in range(NCORES):
        m = core == c
        s_idx = idx_in_chunk[m]
        s_dloc = dstpos[m] % TW
        s_tile = tileid[m]
        s_ch = ch[m]
        s_inv = invdeg[dst[m]]
        # stable order by (chunk, tile)
        key = s_ch * TILES + s_tile
        o = np.argsort(key, kind="stable")
        s_idx, s_dloc, s_inv, key = s_idx[o], s_dloc[o], s_inv[o], key[o]
        # scatter into padded slot arrays
        slot_idx = np.zeros(slots, np.int16)
        slot_dst = np.full(slots, -1.0, np.float32)
        slot_inv = np.zeros(slots, np.float32)
        # position within own (chunk,tile) run
        cnt = counts[c].reshape(-1)            # [NCHUNK*TILES]
        starts = off.reshape(-1)
        # within-key running index
        kcng = np.concatenate([[0], np.cumsum(np.bincount(
            key, minlength=NCHUNK * TILES))])[:-1]
        within = np.arange(key.size) - kcng[key]
        pos_slot = starts[key] + within
        slot_idx[pos_slot] = s_idx.astype(np.int16)
        slot_dst[pos_slot] = s_dloc.astype(np.float32)
        slot_inv[pos_slot] = s_inv
        idx16.append(np.tile(slot_idx.reshape(-1, 16).T, (8, 1)).copy())
        dst32.append(slot_dst.reshape(-1, P).T.copy())
        inv32.append(slot_inv.reshape(-1, P).T.copy())

    return dict(perms=perms, pos_glob=pos_glob, ngroups=ngroups, spans=spans,
                gtot=gtot, slots=slots, idx16=idx16, dst32=dst32, inv32=inv32)


# ---------------------------------------------------------------------------
# device kernel
# ---------------------------------------------------------------------------

def _build_bass(spans, gtot):
    import concourse.bacc as bacc
    import concourse.bass as bass
    import concourse.mybir as mybir
    import concourse.tile as tile
    from concourse.library_config import mlp

    f32 = mybir.dt.float32
    i16 = mybir.dt.int16

    nc = bacc.Bacc("TRN2", target_bir_lowering=False, debug=False,
                   num_devices=NCORES)

    # ---- I/O ----
    x_rep = nc.dram_tensor("x_rep", [NPAD, IN], f32, kind="ExternalInput")
    xT = nc.dram_tensor("xT", [IN, OWNP], f32, kind="ExternalInput")
    eidx = nc.dram_tensor("eidx", [P, gtot * 8], i16, kind="ExternalInput")
    edst = nc.dram_tensor("edst", [P, gtot], f32, kind="ExternalInput")
    einv = nc.dram_tensor("einv", [P, gtot], f32, kind="ExternalInput")
    w_in = [nc.dram_tensor(f"w{i}", [2 * HID, HID], f32, kind="ExternalInput")
            for i in range(3)]
    b_in = [nc.dram_tensor(f"b{i}", [HID], f32, kind="ExternalInput")
            for i in range(3)]
    lw_in = nc.dram_tensor("lin_w", [HID, OUT], f32, kind="ExternalInput")
    lb_in = nc.dram_tensor("lin_b", [OUT], f32, kind="ExternalInput")
    mask_in = [nc.dram_tensor(f"mask{i}", [HID, OWNP], f32,
                              kind="ExternalInput") for i in range(3)]
    outT = nc.dram_tensor("outT", [OUT, OWNP], f32, kind="ExternalOutput")

    iota_np = np.tile(np.arange(TW, dtype=np.float32), (P, 1))
    iota_dram = nc.inline_tensor(iota_np, name="iota_const")
    ident_np = np.eye(P, dtype=np.float32)
    ident_dram = nc.inline_tensor(ident_np, name="ident_const")

    with tile.TileContext(nc, num_cores=NCORES) as tc:
        with (
            tc.tile_pool(name="const", bufs=1) as constp,
            tc.tile_pool(name="meta", bufs=1) as metap,
            tc.tile_pool(name="acc", bufs=1) as accp,
            tc.tile_pool(name="msgs", bufs=2) as msgsp,
            tc.tile_pool(name="spool", bufs=6) as spool,
            tc.tile_pool(name="dense", bufs=3) as densep,
            tc.tile_pool(name="pag", bufs=2, space="PSUM") as pagp,
            tc.tile_pool(name="pz", bufs=2, space="PSUM") as pzp,
            tc.tile_pool(name="ptr", bufs=2, space="PSUM") as ptrp,
            tc.tile_pool(name="dram", bufs=1, space="DRAM") as dramp,
        ):
            nc.gpsimd.load_library(mlp)

            # ---- constants / weights resident in SBUF ----
            iota_t = constp.tile([P, TW], f32)
            nc.sync.dma_start(out=iota_t[:], in_=iota_dram[:, :])
            ident_t = constp.tile([P, P], f32)
            nc.sync.dma_start(out=ident_t[:], in_=ident_dram[:, :])
            wt_t, wb_t, b_t = [], [], []
            for i in range(3):
                wt = constp.tile([HID, HID], f32)
                nc.sync.dma_start(out=wt[:], in_=w_in[i][0:HID, :])
                wb = constp.tile([HID, HID], f32)
                nc.sync.dma_start(out=wb[:], in_=w_in[i][HID:2 * HID, :])
                bt = constp.tile([HID, 1], f32)
                nc.sync.dma_start(out=bt[:], in_=b_in[i][:, None])
                wt_t.append(wt)
                wb_t.append(wb)
                b_t.append(bt)
            lw_t = constp.tile([HID, OUT], f32)
            nc.sync.dma_start(out=lw_t[:], in_=lw_in[:, :])
            lb_t = constp.tile([OUT, 1], f32)
            nc.sync.dma_start(out=lb_t[:], in_=lb_in[:, None])

            # ---- resident edge metadata ----
            idx_t = metap.tile([P, gtot * 8], i16)
            nc.sync.dma_start(out=idx_t[:], in_=eidx[:, :])
            dst_t = metap.tile([P, gtot], f32)
            nc.sync.dma_start(out=dst_t[:], in_=edst[:, :])
            inv_t = metap.tile([P, gtot], f32)
            nc.sync.dma_start(out=inv_t[:], in_=einv[:, :])

            aggT = accp.tile([P, OWNP], f32)

            # ---- DRAM intermediates ----
            hT_buf = [dramp.tile([HID, OWNP], f32, name=f"hT_buf{i}") for i in range(2)]
            h_nodes = [dramp.tile([OWNP, HID], f32, name=f"h_nodes{i}") for i in range(2)]
            h_full = [dramp.tile([NPAD, HID], f32, addr_space="Shared",
                                name=f"h_full{i}") for i in range(2)]

            def aggregate(src_h):
                """aggT[feat, dst] = sum_{e->dst} invdeg * src_h[src_e]."""
                nc.vector.memset(aggT[:], 0.0)
                g0 = 0  # global group counter
                for chk in range(NCHUNK):
                    src_ap = src_h[chk * CHROWS:(chk + 1) * CHROWS, :]
                    # split this chunk's spans into calls of <= GCALL groups
                    # without crossing nothing (tiles may span calls freely)
                    flat = []  # (tile, group-in-tile-first?) -> per group tile id
                    for t, ng in spans[chk]:
                        flat.extend([t] * ng)
                    pos = 0
                    while pos < len(flat):
                        ncall = min(GCALL, len(flat) - pos)
                        msgs = msgsp.tile([P, ncall, P], f32, tag="msgs")
                        nc.gpsimd.dma_gather(
                            out_ap=msgs[:],
                            in_ap=src_ap,
                            idxs_ap=idx_t[:, (g0 + pos) * 8:
                                          (g0 + pos + ncall) * 8],
                            num_idxs=ncall * P,
                            num_idxs_reg=ncall * P,
                            elem_size=P,
                            single_packet=False,
                        )
                        # process groups, batching psum per tile-run
                        j = 0
                        while j < ncall:
                            t = flat[pos + j]
                            j2 = j
                            while j2 < ncall and flat[pos + j2] == t:
                                j2 += 1
                            pt = pagp.tile([P, TW], f32, tag="pagg",
                                           space="PSUM")
                            for k in range(j, j2):
                                g = g0 + pos + k
                                S = spool.tile([P, TW], f32, tag="S")
                                nc.vector.tensor_scalar(
                                    out=S[:], in0=iota_t[:],
                                    scalar1=dst_t[:, g:g + 1],
                                    scalar2=inv_t[:, g:g + 1],
                                    op0=mybir.AluOpType.is_equal,
                                    op1=mybir.AluOpType.mult)
                                nc.tensor.matmul(
                                    out=pt[:], lhsT=msgs[:, k, :], rhs=S[:],
                                    start=(k == j), stop=(k == j2 - 1))
                            nc.vector.tensor_add(
                                out=aggT[:, t * TW:(t + 1) * TW],
                                in0=aggT[:, t * TW:(t + 1) * TW], in1=pt[:])
                            j = j2
                        pos += ncall
                    g0 += len(flat)
                assert g0 == gtot

            def dense_layer(li, hT_in, hT_out, hn_out):
                """z.T = relu(Wt.T @ aggT + Wb.T @ hT_in + b) * mask;
                write feat-major to hT_out and node-major to hn_out."""
                for t in range(DTILES):
                    sl = slice(t * P, (t + 1) * P)
                    hin = densep.tile([HID, P], f32, tag="hin")
                    nc.sync.dma_start(out=hin[:], in_=hT_in[:, sl])
                    mk = densep.tile([HID, P], f32, tag="mk")
                    nc.sync.dma_start(out=mk[:], in_=mask_in[li][:, sl])
                    pz = pzp.tile([HID, P], f32, tag="pz", space="PSUM")
                    nc.tensor.matmul(out=pz[:], lhsT=wt_t[li][:],
                                     rhs=aggT[:, sl], start=True, stop=False)
                    nc.tensor.matmul(out=pz[:], lhsT=wb_t[li][:], rhs=hin[:],
                                     start=False, stop=True)
                    zt = densep.tile([HID, P], f32, tag="zt")
                    nc.scalar.activation(
                        out=zt[:], in_=pz[:],
                        func=mybir.ActivationFunctionType.Relu,
                        bias=b_t[li][:, 0:1], scale=1.0)
                    hn = densep.tile([HID, P], f32, tag="hn")
                    nc.vector.tensor_mul(out=hn[:], in0=zt[:], in1=mk[:])
                    nc.sync.dma_start(out=hT_out[:, sl], in_=hn[:])
                    if hn_out is not None:
                        ptr = ptrp.tile([P, HID], f32, tag="ptr",
                                        space="PSUM")
                        nc.tensor.transpose(out=ptr[:], in_=hn[:],
                                            identity=ident_t[:])
                        hnode = densep.tile([P, HID], f32, tag="hnode")
                        nc.vector.tensor_copy(out=hnode[:], in_=ptr[:])
                        nc.sync.dma_start(out=hn_out[sl, :], in_=hnode[:])

            # ---- layer 1 ----
            aggregate(x_rep)
            dense_layer(0, xT, hT_buf[0], h_nodes[0])
            nc.gpsimd.collective_compute(
                "AllGather", mybir.AluOpType.bypass,
                ins=[h_nodes[0].opt()], outs=[h_full[0].opt()],
                replica_groups=[list(range(NCORES))])

            # ---- layer 2 ----
            aggregate(h_full[0])
            dense_layer(1, hT_buf[0], hT_buf[1], h_nodes[1])
            nc.gpsimd.collective_compute(
                "AllGather", mybir.AluOpType.bypass,
                ins=[h_nodes[1].opt()], outs=[h_full[1].opt()],
                replica_groups=[list(range(NCORES))])

            # ---- layer 3 ----
            aggregate(h_full[1])
            dense_layer(2, hT_buf[1], hT_buf[0], None)

            # ---- final linear: outT[64, dst] = lin_w.T @ h3T + lin_b ----
            for t in range(DTILES):
                sl = slice(t * P, (t + 1) * P)
                h3 = densep.tile([HID, P], f32, tag="hin")
                nc.sync.dma_start(out=h3[:], in_=hT_buf[0][:, sl])
                po = pzp.tile([OUT, P], f32, tag="po", space="PSUM")
                nc.tensor.matmul(out=po[:], lhsT=lw_t[:], rhs=h3[:],
                                 start=True, stop=True)
                ot = densep.tile([OUT, P], f32, tag="ot")
                nc.vector.tensor_scalar(
                    out=ot[:], in0=po[:], scalar1=lb_t[:, 0:1], scalar2=None,
                    op0=mybir.AluOpType.add)
                nc.sync.dma_start(out=outT[:, sl], in_=ot[:])

    nc.compile()
    return nc


# ---------------------------------------------------------------------------
# entry point
# ---------------------------------------------------------------------------

def kernel(x, edge, w0, b0, w1, b1, w2, b2, lin_w, lin_b, size_src, size_dst,
           **_unused):
    t_start = time.time()
    x = np.ascontiguousarray(np.asarray(x, dtype=np.float32))
    edge = np.asarray(edge)
    n_dst = int(size_dst)

    masks = _dropout_masks()
    pp = _preprocess(edge)
    perms, pos_glob = pp["perms"], pp["pos_glob"]

    # padded replicated x (row pos_glob[n] = x[n])
    x_rep = np.zeros((NPAD, IN), np.float32)
    x_rep[pos_glob] = x

    in_maps = []
    for c in range(NCORES):
        nodes = c * OWN + perms[c]          # original node ids, padded order
        xT_c = np.zeros((IN, OWNP), np.float32)
        xT_c[:, :OWN] = x[nodes].T
        m = {
            "x_rep": x_rep, "xT": np.ascontiguousarray(xT_c),
            "eidx": pp["idx16"][c], "edst": pp["dst32"][c],
            "einv": pp["inv32"][c],
            "w0": np.asarray(w0, np.float32), "b0": np.asarray(b0, np.float32),
            "w1": np.asarray(w1, np.float32), "b1": np.asarray(b1, np.float32),
            "w2": np.asarray(w2, np.float32), "b2": np.asarray(b2, np.float32),
            "lin_w": np.asarray(lin_w, np.float32),
            "lin_b": np.asarray(lin_b, np.float32),
        }
        for i in range(3):
            mT = np.zeros((HID, OWNP), np.float32)
            mT[:, :OWN] = masks[i][nodes].T
            m[f"mask{i}"] = np.ascontiguousarray(mT)
        in_maps.append(m)

    print(f"[kernel] host prep {time.time() - t_start:.1f}s", flush=True)
    t0 = time.time()
    nc = _build_bass(pp["spans"], pp["gtot"])
    print(f"[kernel] bass build {time.time() - t0:.1f}s", flush=True)

    from concourse.bass_utils import run_bass_kernel_spmd
    trace = bool(int(os.environ.get("GNN_TRACE", "0")))
    t0 = time.time()
    res = run_bass_kernel_spmd(nc, in_maps, core_ids=list(range(NCORES)),
                               trace=trace)
    print(f"[kernel] device run {time.time() - t0:.1f}s", flush=True)
    if res.exec_time_ns is not None:
        print(f"HW exec time: {res.exec_time_ns} ns", flush=True)

    out = np.empty((N, OUT), np.float32)
    for c in range(NCORES):
        nodes = c * OWN + perms[c]
        out[nodes] = res.results[c]["outT"][:, :OWN].T
    return out[:n_dst]


if __name__ == "__main__":
    # quick self-run against the jax reference
    import jax

    with jax.default_device(jax.devices("cpu")[0]):
        import reference

        inputs = reference.setup_inputs()
        expected = np.asarray(reference.reference(**inputs))
    actual = kernel(**{k: np.asarray(v) if hasattr(v, "shape") else v
                       for k, v in inputs.items()})
    err = np.abs(actual - expected).max()
    denom = np.abs(expected).max()
    print(f"abs err {err:.3e}  (absmax {denom:.3e}, rel {err / denom:.3e})")
